# revision 1
# baseline (speedup 1.0000x reference)
"""GroupPearson Trainium2 kernel, v5.1: sort-by-group sharding + streaming
reduce split across DVE and ACT.

Sharding strategy: the host sorts elements by group id (segment-sharded
distribution), pads each group to a fixed stride FG, and lays groups out so
each of the 8 cores owns 512 whole groups and each partition owns 4 whole
groups (split into 2 half-chunks for pipelining). Three per-element bf16
streams are shipped (x, y, x*y); group counts fall out of the host's
bincount used for the padded layout.

Device, per half-chunk [P, 3, FG2]:
  DVE: one multi-dim tensor_reduce over (x, xy)        -> sx, sxy (fp32)
  ACT: Square+accum(x) -> sxx, Square+accum(y) -> syy,
       Copy+accum(y) -> sy   (fp32 accumulation)
Output [P, NCH*5] fp32; host sums half-chunk partials in float64 and
finishes the correlation exactly as the reference does.
"""

import numpy as np
import ml_dtypes

P = 128
G = 4096
NST = 3                      # shipped streams: x, xy, y
NSUM = 5                     # sx, sxy, sxx, syy, sy

N_TOTAL = 16_777_216
N_CORES = 8
GPC = G // N_CORES           # 512 groups per core
SLOTS = GPC // P             # 4 groups per partition
HALVES = 1
NCH = SLOTS * HALVES         # 8 chunks per core


def build_nc(FG2, n_devices=N_CORES):
    from concourse import mybir, tile, bacc
    from contextlib import ExitStack

    dt = mybir.dt
    AF = mybir.ActivationFunctionType
    OP = mybir.AluOpType

    nc = bacc.Bacc("TRN2", target_bir_lowering=False, debug=False,
                   num_devices=n_devices)
    # stream order per chunk: [x, xy, y]
    v_d = nc.dram_tensor("v", [P, NCH, NST, FG2], dt.bfloat16,
                         kind="ExternalInput").ap()
    o_d = nc.dram_tensor("o", [P, NCH * NSUM], dt.float32,
                         kind="ExternalOutput").ap()

    with tile.TileContext(nc) as tc, ExitStack() as ctx:
        out_pool = ctx.enter_context(tc.tile_pool(name="out", bufs=1))
        io_pool = ctx.enter_context(tc.tile_pool(name="io", bufs=3))
        scr_pool = ctx.enter_context(tc.tile_pool(name="scr", bufs=2))

        outs = out_pool.tile([P, NCH * NSUM], dt.float32)

        for c in range(NCH):
            t = io_pool.tile([P, NST * FG2], dt.bfloat16, tag="t")
            t3 = t[:, :].rearrange("p (n f) -> p n f", f=FG2)
            nc.sync.dma_start(out=t3, in_=v_d[:, c, :, :])
            ob = c * NSUM
            # DVE: sx, sxy in one multi-dim reduce over streams 0..1
            nc.vector.tensor_reduce(
                outs[:, ob:ob + 2], t3[:, 0:2, :],
                mybir.AxisListType.X, OP.add)
            # ACT: sxx, syy, sy with fused fp32 accumulation
            scr = scr_pool.tile([P, FG2], dt.bfloat16, tag="scr")
            nc.scalar.activation(scr[:, :], t3[:, 0, :], AF.Square,
                                 accum_out=outs[:, ob + 2:ob + 3])
            scr2 = scr_pool.tile([P, FG2], dt.bfloat16, tag="scr2")
            nc.scalar.activation(scr2[:, :], t3[:, 2, :], AF.Square,
                                 accum_out=outs[:, ob + 3:ob + 4])
            scr3 = scr_pool.tile([P, FG2], dt.bfloat16, tag="scr3")
            nc.scalar.activation(scr3[:, :], t3[:, 2, :], AF.Copy,
                                 accum_out=outs[:, ob + 4:ob + 5])

        nc.sync.dma_start(out=o_d[:, :], in_=outs[:, :])

    nc.compile()
    return nc


def host_sort_pad(pred, exp, group):
    """Sorted+padded [NST, G*FG] bf16 streams (x, xy, y) + sizes, FG."""
    bf = ml_dtypes.bfloat16
    x = np.asarray(exp, dtype=np.float32)
    y = np.asarray(pred, dtype=np.float32)
    g = np.asarray(group).astype(np.int32)
    n = g.shape[0]

    sizes = np.bincount(g, minlength=G)
    FG = int(np.ceil(max(int(sizes.max()), 1) / 256.0) * 256)
    order = np.argsort(g, kind="stable")
    gs = g[order].astype(np.int64)
    starts = np.zeros(G, dtype=np.int64)
    starts[1:] = np.cumsum(sizes)[:-1]
    pos = np.arange(n, dtype=np.int64) - starts[gs]
    dst = gs * FG + pos

    v3s = np.zeros((NST, G * FG), dtype=bf)
    v3s[0, dst] = x.astype(bf)[order]
    v3s[1, dst] = (x * y).astype(bf)[order]
    v3s[2, dst] = y.astype(bf)[order]
    return v3s, sizes.astype(np.float64), FG


def _finish_host(S):
    n, sx, sy, sxy, sxx, syy = S
    n_safe = np.where(n > 0, n, 1.0)
    mx = sx / n_safe
    my = sy / n_safe
    cov = sxy / n_safe - mx * my
    var_x = sxx / n_safe - mx * mx
    var_y = syy / n_safe - my * my
    denom = np.sqrt(np.maximum(var_x * var_y, 0.0))
    corr = np.where(denom > 0, cov / np.where(denom > 0, denom, 1.0), 0.0)
    corr_pearson = np.sum(corr * n) / np.sum(n)
    return np.float32(-corr_pearson)


_NC_CACHE = {}


def _get_nc(FG2):
    if FG2 not in _NC_CACHE:
        _NC_CACHE[FG2] = build_nc(FG2)
    return _NC_CACHE[FG2]


def kernel(pred, exp, group, num_groups, _trace=False):
    from concourse.bass_utils import run_bass_kernel_spmd

    pred = np.asarray(pred)
    exp = np.asarray(exp)
    group = np.asarray(group)
    assert pred.shape == (N_TOTAL,)

    v3s, sizes, FG = host_sort_pad(pred, exp, group)
    FG2 = FG // HALVES
    nc = _get_nc(FG2)

    # [NST, G*FG] -> per core [P, NCH, NST, FG2]
    vr = v3s.reshape(NST, N_CORES, P, SLOTS, HALVES, FG2)
    in_maps = []
    for i in range(N_CORES):
        # -> [P, SLOTS, HALVES, NST, FG2] -> [P, NCH, NST, FG2]
        vi = np.ascontiguousarray(vr[:, i].transpose(1, 2, 3, 0, 4))
        in_maps.append({"v": vi.reshape(P, NCH, NST, FG2)})

    res = run_bass_kernel_spmd(nc, in_maps, list(range(N_CORES)),
                               trace=_trace)

    S = np.zeros((6, G), dtype=np.float64)
    S[0] = sizes
    for i in range(N_CORES):
        o = res.results[i]["o"].astype(np.float64)
        o = o.reshape(P, SLOTS, HALVES, NSUM).sum(axis=2)  # [P, SLOTS, NSUM]
        # group g = 512*i + 4*p + s ; sums order: sx, sxy, sxx, syy, sy
        blk = o.transpose(2, 0, 1).reshape(NSUM, GPC)
        S[1, GPC * i:GPC * (i + 1)] = blk[0]
        S[3, GPC * i:GPC * (i + 1)] = blk[1]
        S[4, GPC * i:GPC * (i + 1)] = blk[2]
        S[5, GPC * i:GPC * (i + 1)] = blk[3]
        S[2, GPC * i:GPC * (i + 1)] = blk[4]
    out = _finish_host(S)
    if _trace:
        return out, res
    return out



# revision 3
# speedup vs baseline: 1.3011x; 1.3011x over previous
"""GroupPearson Trainium2 kernel, v6: PE diagonal-matmul segment reduction.

Sharding: host sorts elements by group id, pads each group to FG = T*128
elements, and ships two bf16 streams (x=exp, y=pred) per core in a
"slab" layout: v[k, w, t, c] = element (128*t + k) of group (512*core +
128*w + c).  Each core owns 512 groups = 4 psum-windows of 128 groups.

Device, per window w and slab t (stationary = slab, groups as columns):
  MM(psumA, lhsT=x_t, rhs=y_t)   -> diag(psumA)[g] += sum_k x[k,g]*y[k,g]
  MM(psumB, lhsT=x_t, rhs=x_t)   -> diag = sum x^2
  MM(psumC, lhsT=x_t, rhs=ones)  -> psumC[g,0] += sum_k x[k,g]
  MM(psumD, lhsT=y_t, rhs=y_t)   -> diag = sum y^2
  MM(psumE, lhsT=y_t, rhs=ones)  -> psumE[g,0] += sum_k y[k,g]
accumulated over t in PSUM (fp32).  After T slabs, DVE extracts the
diagonals with an identity-mask scalar_tensor_tensor reduce and copies
the ones-columns.  Output [P, NW*5] f32; host finishes the correlation.
"""

import numpy as np
import ml_dtypes

P = 128
G = 4096
N_CORES = 8
GPC = G // N_CORES           # 512 groups per core
NW = GPC // P                # 4 psum windows of 128 groups
NSUM = 5


def build_nc(T, n_devices=N_CORES):
    from concourse import mybir, tile, bacc
    from contextlib import ExitStack

    dt = mybir.dt
    OP = mybir.AluOpType

    nc = bacc.Bacc("TRN2", target_bir_lowering=False, debug=False,
                   num_devices=n_devices)
    vx_d = nc.dram_tensor("vx", [P, NW, T * P], dt.bfloat16,
                          kind="ExternalInput").ap()
    vy_d = nc.dram_tensor("vy", [P, NW, T * P], dt.bfloat16,
                          kind="ExternalInput").ap()
    id_d = nc.dram_tensor("ident", [P, P], dt.bfloat16,
                          kind="ExternalInput").ap()
    o_d = nc.dram_tensor("o", [P, NW * NSUM], dt.float32,
                         kind="ExternalOutput").ap()

    with tile.TileContext(nc) as tc, ExitStack() as ctx:
        const_pool = ctx.enter_context(tc.tile_pool(name="const", bufs=1))
        out_pool = ctx.enter_context(tc.tile_pool(name="out", bufs=1))
        io_pool = ctx.enter_context(tc.tile_pool(name="io", bufs=3))
        scr_pool = ctx.enter_context(tc.tile_pool(name="scr", bufs=2))
        ps_pool = ctx.enter_context(tc.psum_pool(name="ps", bufs=2))

        ones = const_pool.tile([P, 1], dt.bfloat16)
        nc.vector.memset(ones[:, :], 1.0)
        ident = const_pool.tile([P, P], dt.bfloat16)
        nc.sync.dma_start(out=ident[:, :], in_=id_d)

        outs = out_pool.tile([P, NW * NSUM], dt.float32)

        for w in range(NW):
            xw = io_pool.tile([P, T * P], dt.bfloat16, tag="xw")
            yw = io_pool.tile([P, T * P], dt.bfloat16, tag="yw")
            nc.sync.dma_start(out=xw[:, :], in_=vx_d[:, w, :])
            nc.sync.dma_start(out=yw[:, :], in_=vy_d[:, w, :])
            x3 = xw[:, :].rearrange("p (t c) -> p t c", c=P)
            y3 = yw[:, :].rearrange("p (t c) -> p t c", c=P)

            # one PSUM bank: A | B | D | C | E
            ps = ps_pool.tile([P, 3 * P + 2], dt.float32, tag="ps")
            psA = ps[:, 0:P]
            psB = ps[:, P:2 * P]
            psD = ps[:, 2 * P:3 * P]
            psC = ps[:, 3 * P:3 * P + 1]
            psE = ps[:, 3 * P + 1:3 * P + 2]

            # start=True clears has_written for the WHOLE psum bank, so only
            # the very first matmul into the bank may carry it; later first
            # writes to each region overwrite via has_written=0.
            for t in range(T):
                xt = x3[:, t, :]
                yt = y3[:, t, :]
                st = (t == 0)
                sp = (t == T - 1)
                nc.tensor.matmul(psA, lhsT=xt, rhs=yt, start=st, stop=sp,
                                 skip_group_check=True)
                nc.tensor.matmul(psB, lhsT=xt, rhs=xt, start=False, stop=sp,
                                 skip_group_check=True)
                nc.tensor.matmul(psC, lhsT=xt, rhs=ones[:, :], start=False,
                                 stop=sp, skip_group_check=True)
                nc.tensor.matmul(psD, lhsT=yt, rhs=yt, start=False, stop=sp,
                                 skip_group_check=True)
                nc.tensor.matmul(psE, lhsT=yt, rhs=ones[:, :], start=False,
                                 stop=sp, skip_group_check=True)

            ob = w * NSUM
            # diag extraction: accum_out = sum_n psum[m, n] * I[m, n]
            for j, psX in enumerate((psA, psB, psD)):
                scr = scr_pool.tile([P, P], dt.float32, tag=f"scr{j}")
                nc.vector.scalar_tensor_tensor(
                    scr[:, :], psX, 1.0, ident[:, :], OP.mult, OP.mult,
                    accum_out=outs[:, ob + j:ob + j + 1])
            nc.vector.tensor_copy(outs[:, ob + 3:ob + 4], psC)
            nc.vector.tensor_copy(outs[:, ob + 4:ob + 5], psE)

        nc.sync.dma_start(out=o_d[:, :], in_=outs[:, :])

    nc.compile()
    return nc


def host_layout(pred, exp, group):
    """Sorted+padded slab layout: per-core [P, NW, T*P] bf16 x and y."""
    bf = ml_dtypes.bfloat16
    x = np.asarray(exp, dtype=np.float32)
    y = np.asarray(pred, dtype=np.float32)
    g = np.asarray(group).astype(np.int32)
    n = g.shape[0]

    sizes = np.bincount(g, minlength=G)
    T = max(int(np.ceil(int(sizes.max()) / P)), 1)
    FG = T * P
    order = np.argsort(g, kind="stable")
    gs = g[order].astype(np.int64)
    starts = np.zeros(G, dtype=np.int64)
    starts[1:] = np.cumsum(sizes)[:-1]
    pos = np.arange(n, dtype=np.int64) - starts[gs]
    dst = gs * FG + pos

    out = []
    for v in (x, y):
        pad = np.zeros(G * FG, dtype=bf)
        pad[dst] = v.astype(bf)[order]
        # [core, w, c, t, k] -> [core, k, w, t, c]
        a = pad.reshape(N_CORES, NW, P, T, P)
        a = np.ascontiguousarray(a.transpose(0, 4, 1, 3, 2))
        out.append(a.reshape(N_CORES, P, NW, T * P))
    return out[0], out[1], sizes.astype(np.float64), T


def _finish_host(S):
    n, sx, sy, sxy, sxx, syy = S
    n_safe = np.where(n > 0, n, 1.0)
    mx = sx / n_safe
    my = sy / n_safe
    cov = sxy / n_safe - mx * my
    var_x = sxx / n_safe - mx * mx
    var_y = syy / n_safe - my * my
    denom = np.sqrt(np.maximum(var_x * var_y, 0.0))
    corr = np.where(denom > 0, cov / np.where(denom > 0, denom, 1.0), 0.0)
    corr_pearson = np.sum(corr * n) / np.sum(n)
    return np.float32(-corr_pearson)


_NC_CACHE = {}


def _get_nc(T):
    if T not in _NC_CACHE:
        _NC_CACHE[T] = build_nc(T)
    return _NC_CACHE[T]


def kernel(pred, exp, group, num_groups, _trace=False):
    from concourse.bass_utils import run_bass_kernel_spmd

    pred = np.asarray(pred)
    exp = np.asarray(exp)
    group = np.asarray(group)

    vx, vy, sizes, T = host_layout(pred, exp, group)
    nc = _get_nc(T)

    ident = np.eye(P, dtype=ml_dtypes.bfloat16)
    in_maps = [{"vx": vx[i], "vy": vy[i], "ident": ident}
               for i in range(N_CORES)]

    res = run_bass_kernel_spmd(nc, in_maps, list(range(N_CORES)),
                               trace=_trace)

    S = np.zeros((6, G), dtype=np.float64)
    S[0] = sizes
    for i in range(N_CORES):
        o = res.results[i]["o"].astype(np.float64)  # [P, NW*5]
        o = o.reshape(P, NW, NSUM)                  # partition=group-in-window
        # group = 512*i + 128*w + m  ; stats order: sxy, sxx, syy, sx, sy
        blk = o.transpose(2, 1, 0).reshape(NSUM, GPC)  # [5, w*128+m]
        S[3, GPC * i:GPC * (i + 1)] = blk[0]
        S[4, GPC * i:GPC * (i + 1)] = blk[1]
        S[5, GPC * i:GPC * (i + 1)] = blk[2]
        S[1, GPC * i:GPC * (i + 1)] = blk[3]
        S[2, GPC * i:GPC * (i + 1)] = blk[4]
    out = _finish_host(S)
    if _trace:
        return out, res
    return out


# revision 5
# speedup vs baseline: 1.5553x; 1.1954x over previous
"""GroupPearson Trainium2 kernel, v7: PE diagonal-matmul segment reduction.

Sharding: host sorts elements by group id, pads each group to FG = T*128
elements, and ships two fp8(e4m3) streams (x=exp, y=pred) per core in a
"slab" layout with a baked ones-column: v[k, w, t, c] holds element
(128*t + k) of group (512*core + 128*w + c) for c < 128, and 1.0 for
c == 128.  Each core owns 512 groups = 4 psum-windows of 128 groups.

Device, per window w and slab t (stationary = x or y slab, groups as
columns; moving = 129-wide slab including the ones column):
  MM(psA, lhsT=x_t, rhs=[y_t|1]) -> diag += sum_k x*y ; col128 += sum_k x
  MM(psB, lhsT=x_t, rhs=x_t)     -> diag += sum_k x*x
  MM(psD, lhsT=y_t, rhs=[y_t|1]) -> diag += sum_k y*y ; col128 += sum_k y
accumulated over t in PSUM (fp32; only the first matmul into a bank may
carry start=True - start clears has_written for the whole bank).  DVE
extracts the diagonals with an identity-mask scalar_tensor_tensor
reduce and copies the ones-columns.  Output [P, NW*5] f32; host
finishes the correlation in float64.
"""

import numpy as np
import ml_dtypes

P = 128
C = 129                      # slab columns: 128 groups + ones column
G = 4096
N_CORES = 8
GPC = G // N_CORES           # 512 groups per core
NW = GPC // P                # 4 psum windows of 128 groups
NSUM = 5
NCK = 4                      # dma chunks per stream-window
USE_FP8 = True


def _enable_ldw_opt():
    # kept for reference; walrus ldw-opt is incompatible with the
    # pre-split InstLdweights that tile legalization emits.
    pass


def build_nc(T, n_devices=N_CORES):
    from concourse import mybir, tile, bacc
    from contextlib import ExitStack

    dt = mybir.dt
    OP = mybir.AluOpType
    ddt = dt.float8e4 if USE_FP8 else dt.bfloat16

    nc = bacc.Bacc("TRN2", target_bir_lowering=False, debug=False,
                   num_devices=n_devices)
    vx_d = nc.dram_tensor("vx", [P, NW, T * C], ddt,
                          kind="ExternalInput").ap()
    vy_d = nc.dram_tensor("vy", [P, NW, T * C], ddt,
                          kind="ExternalInput").ap()
    id_d = nc.dram_tensor("ident", [P, P], dt.bfloat16,
                          kind="ExternalInput").ap()
    o_d = nc.dram_tensor("o", [P, NW * NSUM], dt.float32,
                         kind="ExternalOutput").ap()

    with tile.TileContext(nc) as tc, ExitStack() as ctx:
        const_pool = ctx.enter_context(tc.tile_pool(name="const", bufs=1))
        out_pool = ctx.enter_context(tc.tile_pool(name="out", bufs=1))
        io_pool = ctx.enter_context(tc.tile_pool(name="io", bufs=3))
        scr_pool = ctx.enter_context(tc.tile_pool(name="scr", bufs=2))
        ps_pool = ctx.enter_context(tc.psum_pool(name="ps", bufs=2))

        ident = const_pool.tile([P, P], dt.bfloat16)
        nc.sync.dma_start(out=ident[:, :], in_=id_d)

        outs = out_pool.tile([P, NW * NSUM], dt.float32)

        CK = T * C // NCK if (T * C) % NCK == 0 else None
        for w in range(NW):
            xw = io_pool.tile([P, T * C], ddt, tag="xw")
            yw = io_pool.tile([P, T * C], ddt, tag="yw")
            if CK is not None:
                for k in range(NCK):
                    sl = slice(CK * k, CK * (k + 1))
                    nc.sync.dma_start(out=xw[:, sl], in_=vx_d[:, w, sl])
                    nc.sync.dma_start(out=yw[:, sl], in_=vy_d[:, w, sl])
            else:
                nc.sync.dma_start(out=xw[:, :], in_=vx_d[:, w, :])
                nc.sync.dma_start(out=yw[:, :], in_=vy_d[:, w, :])
            x3 = xw[:, :].rearrange("p (t c) -> p t c", c=C)
            y3 = yw[:, :].rearrange("p (t c) -> p t c", c=C)

            # one PSUM bank: A | B | D  (each [P, C])
            ps = ps_pool.tile([P, 3 * C], dt.float32, tag="ps")
            psA = ps[:, 0:C]
            psB = ps[:, C:2 * C]
            psD = ps[:, 2 * C:3 * C]

            for t in range(T):
                xs = x3[:, t, 0:P]          # stationary: 128 group columns
                ys = y3[:, t, 0:P]
                xm = x3[:, t, :]            # moving: 129 cols (with ones)
                ym = y3[:, t, :]
                st = (t == 0)
                sp = (t == T - 1)
                nc.tensor.matmul(psA, lhsT=xs, rhs=ym, start=st, stop=sp,
                                 skip_group_check=True)
                nc.tensor.matmul(psB, lhsT=xs, rhs=xm, start=False, stop=sp,
                                 skip_group_check=True)
                nc.tensor.matmul(psD, lhsT=ys, rhs=ym, start=False, stop=sp,
                                 skip_group_check=True)

            ob = w * NSUM
            # diag extraction: accum_out = sum_n psum[m, n] * I[m, n]
            for j, psX in enumerate((psA, psB, psD)):
                scr = scr_pool.tile([P, P], dt.float32, tag=f"scr{j}")
                nc.vector.scalar_tensor_tensor(
                    scr[:, :], psX[:, 0:P], 1.0, ident[:, :], OP.mult,
                    OP.mult, accum_out=outs[:, ob + j:ob + j + 1])
            # ones-columns: psA col 128 = sx ; psD col 128 = sy
            nc.vector.tensor_copy(outs[:, ob + 3:ob + 4], psA[:, P:P + 1])
            nc.vector.tensor_copy(outs[:, ob + 4:ob + 5], psD[:, P:P + 1])

        nc.sync.dma_start(out=o_d[:, :], in_=outs[:, :])

    nc.compile()
    return nc


def host_layout(pred, exp, group):
    """Sorted+padded slab layout: per-core [P, NW, T*C] streams x and y."""
    from concourse import mybir
    ddt = mybir.dt.np(mybir.dt.float8e4) if USE_FP8 else ml_dtypes.bfloat16

    x = np.asarray(exp, dtype=np.float32)
    y = np.asarray(pred, dtype=np.float32)
    g = np.asarray(group).astype(np.int32)
    n = g.shape[0]

    sizes = np.bincount(g, minlength=G)
    T = max(int(np.ceil(int(sizes.max()) / P)), 1)
    FG = T * P
    order = np.argsort(g, kind="stable")
    gs = g[order].astype(np.int64)
    starts = np.zeros(G, dtype=np.int64)
    starts[1:] = np.cumsum(sizes)[:-1]
    pos = np.arange(n, dtype=np.int64) - starts[gs]
    dst = gs * FG + pos

    out = []
    for v in (x, y):
        pad = np.zeros(G * FG, dtype=ddt)
        pad[dst] = v.astype(ddt)[order]
        # [core, w, c, t, k] -> [core, k, w, t, c], then append ones col
        a = pad.reshape(N_CORES, NW, P, T, P)
        a = np.ascontiguousarray(a.transpose(0, 4, 1, 3, 2))
        b = np.empty((N_CORES, P, NW, T, C), dtype=ddt)
        b[..., :P] = a
        b[..., P] = ddt(1.0)
        out.append(b.reshape(N_CORES, P, NW, T * C))
    return out[0], out[1], sizes.astype(np.float64), T


def _finish_host(S):
    n, sx, sy, sxy, sxx, syy = S
    n_safe = np.where(n > 0, n, 1.0)
    mx = sx / n_safe
    my = sy / n_safe
    cov = sxy / n_safe - mx * my
    var_x = sxx / n_safe - mx * mx
    var_y = syy / n_safe - my * my
    denom = np.sqrt(np.maximum(var_x * var_y, 0.0))
    corr = np.where(denom > 0, cov / np.where(denom > 0, denom, 1.0), 0.0)
    corr_pearson = np.sum(corr * n) / np.sum(n)
    return np.float32(-corr_pearson)


_NC_CACHE = {}


def _get_nc(T):
    if T not in _NC_CACHE:
        _NC_CACHE[T] = build_nc(T)
    return _NC_CACHE[T]


def kernel(pred, exp, group, num_groups, _trace=False):
    from concourse.bass_utils import run_bass_kernel_spmd

    pred = np.asarray(pred)
    exp = np.asarray(exp)
    group = np.asarray(group)

    vx, vy, sizes, T = host_layout(pred, exp, group)
    nc = _get_nc(T)

    ident = np.eye(P, dtype=ml_dtypes.bfloat16)
    in_maps = [{"vx": vx[i], "vy": vy[i], "ident": ident}
               for i in range(N_CORES)]

    res = run_bass_kernel_spmd(nc, in_maps, list(range(N_CORES)),
                               trace=_trace)

    S = np.zeros((6, G), dtype=np.float64)
    S[0] = sizes
    for i in range(N_CORES):
        o = res.results[i]["o"].astype(np.float64)  # [P, NW*5]
        o = o.reshape(P, NW, NSUM)                  # partition=group-in-window
        # group = 512*i + 128*w + m  ; stats order: sxy, sxx, syy, sx, sy
        blk = o.transpose(2, 1, 0).reshape(NSUM, GPC)  # [5, w*128+m]
        S[3, GPC * i:GPC * (i + 1)] = blk[0]
        S[4, GPC * i:GPC * (i + 1)] = blk[1]
        S[5, GPC * i:GPC * (i + 1)] = blk[2]
        S[1, GPC * i:GPC * (i + 1)] = blk[3]
        S[2, GPC * i:GPC * (i + 1)] = blk[4]
    out = _finish_host(S)
    if _trace:
        return out, res
    return out
